# revision 53
# baseline (speedup 1.0000x reference)
"""Trainium2 Bass kernel for nn_CNN_ODE — v4 (fp8-DR conv + RK4-1).

v3 structure (RK4-1, staggered conv/enc emission, z-space ODE) with the
conv stage moved to fp8e4m3 DoubleRow matmuls: x and conv weights are
quantized to e4m3 (emulated end-to-end rel err ~1.2e-2 vs the 2e-2 gate),
and each feature tile's two adjacent K=128 input chunks are summed in one
DoubleRow matmul (K=256 @ 0.5 cycles/col — 4x the f16 rate).  This cuts
conv PE time ~4x and halves the x DMA/staging bytes.  enc1 must stay f16
(fp8 h+enc1w measured 2.03e-2 — over the gate).  The enc1/regressor ReLUs
move to DVE tensor_scalar (add-bias, max 0) to shrink the ACT-engine
stream, which is the device bottleneck (SiLU over 1440x8192 elements).

`build_nc(n_passes=K)` unrolls K full passes in one NEFF so steady-state
per-pass device time can be measured with host dispatch + input staging
amortized (test.py uses this for timing; kernel() uses one pass).

Scheduling: per slot the emission is [silu r][conv r+2][cover: enc1 r-2 /
ODE pops] so the in-order PE stream never stalls head-of-line with
runnable cover work queued behind it; the previous sg's enc1 tail (rr=10,
11) is emitted as cover in the next sg's first two slots; conv PSUM is two
static [128,1024] tiles (manual double-buffer via region deps — a single
3D [128,2,1024] tile gets coarse deps and serializes; pool-rotated tiles
add SP TileRelease/alloc hops).

Measured wall tracks TOTAL instruction count (~150 ns/instruction; the PE
stream of MATMUL+LDWEIGHTS pairs is ~70% of it), not per-engine busy time
— so the winning lever is instruction-count reduction: x is staged with 4
extra 64-offset chunks (OFF_BASES) so the two feature tiles whose input
span crosses 3 aligned chunks (r3, r6) become single DoubleRow pairs,
cutting conv to 12 pairs/sg (−64 PE instructions/pass ≈ −10 us).  N=1024
matmuls are architecturally impossible (ISA s3d3_mm_num_elements caps the
moving dim at 512) and walrus never dedupes identical consecutive
LDWEIGHTS, so 2x512 halves each reload weights.  ~188 us/pass measured.
"""

import numpy as np

import concourse.bass as bass
import concourse.bacc as bacc
import concourse.mybir as mybir
from concourse.tile import TileContext
from concourse.bass_utils import run_bass_kernel_spmd

F16 = mybir.dt.float16
F32 = mybir.dt.float32
F8 = mybir.dt.float8e4
F8NP = mybir.dt.np(F8)
DR = mybir.MatmulPerfMode.DoubleRow
AF = mybir.ActivationFunctionType
ALU = mybir.AluOpType

N_CORES = 8
B_TOTAL = 65536
SEQ, IN_DIM = 40, 24
N_STEPS = 1                     # RK4 steps
BPC = B_TOTAL // N_CORES        # 8192 samples/core
HB = BPC // 2                   # stacked tile width (4096)
NSG = 8                         # super-groups of 1024 samples
NW = 4                          # ODE waves of 1024 cols
NCH = 8                         # 512-col chunk columns
NFT = 12                        # conv output feature tiles of 128
NCK = 8                         # 128-row input chunks (1024 padded rows)


def _conv_pieces():
    pieces = []
    for r in range(NFT):
        fr0, fr1 = 128 * r, min(128 * r + 127, 1439)
        s0, s1 = fr0 // 36, fr1 // 36
        a, b = 24 * s0, min(24 * (s1 + 3) - 1, 1007)
        for k in range(a // 128, b // 128 + 1):
            pieces.append((r, k))
    return pieces


# Extra chunks in the xt tile holding x rows at a 64-offset grid, so the
# two feature tiles whose 3-chunk input span breaks the aligned grid (r3:
# rows 240-407, r6: rows 504-647) become single DoubleRow pairs.  Chunk
# tile-index NCK+i holds padded-x rows OFF_BASES[i]..+127.
OFF_BASES = [192, 320, 448, 576]
NCKT = NCK + len(OFF_BASES)     # chunks in the xt tile


def _conv_pairs():
    # DoubleRow pairs: (r, tile_chunk, valid_halves, row_base) — one matmul
    # covers tile chunks (tile_chunk, tile_chunk+1) = padded-x rows
    # row_base..row_base+255; halves absent from `valid` get zero weights.
    by_r = {}
    for r, k in _conv_pieces():
        by_r.setdefault(r, []).append(k)
    pairs = []
    for r in range(NFT):
        ks = by_r[r]
        if len(ks) == 3:
            # 3-chunk span: use the 64-offset grid (one pair)
            fr0, fr1 = 128 * r, min(128 * r + 127, 1439)
            a = 24 * (fr0 // 36)
            b = min(24 * (fr1 // 36 + 3) - 1, 1007)
            for i in range(0, len(OFF_BASES), 2):
                if OFF_BASES[i] <= a and b <= OFF_BASES[i] + 255:
                    pairs.append((r, NCK + i, (0, 1), OFF_BASES[i]))
                    break
            else:
                raise AssertionError(f"no offset pair covers r={r}")
            continue
        i = 0
        while i < len(ks):
            if i + 1 < len(ks) and ks[i + 1] == ks[i] + 1:
                pairs.append((r, ks[i], (0, 1), 128 * ks[i]))
                i += 2
            else:
                c = ks[i]
                k0 = c if c < NCK - 1 else c - 1
                pairs.append((r, k0, (c - k0,), 128 * k0))
                i += 1
    return pairs


def make_consts(inputs, steps=N_STEPS):
    f16 = np.float16
    g = {k: np.asarray(v, dtype=np.float64) for k, v in inputs.items() if k != "x"}
    dt = float(g["t_span"][1] - g["t_span"][0]) / steps
    W1, b1 = g["ode1_w"], g["ode1_b"]
    W2, b2 = g["ode2_w"], g["ode2_b"]
    e2b = g["enc2_b"]
    M = W1 @ W2
    cvec = W1 @ b2
    wz = W1 @ e2b                  # enc2 bias folded into z-space
    c = {}

    def bd(X):
        o = np.zeros((128, 128))
        o[0:64, 0:64] = X
        o[64:128, 64:128] = X
        return o

    c["A"] = bd((dt / 2 * M).T).astype(f16)
    c["C"] = bd((dt * M).T).astype(f16)
    c["D"] = bd((dt / 6 * M).T).astype(f16)
    c["negC"] = bd((-dt * M).T).astype(f16)
    c["w1t"] = bd(W1.T).astype(f16)

    betas = np.zeros((64, 3 * steps))
    for n in range(steps):
        base = b1 + wz + n * dt * cvec
        betas[:, 3 * n + 0] = base
        betas[:, 3 * n + 1] = base + dt / 2 * cvec
        betas[:, 3 * n + 2] = base + dt * cvec
    c["betas"] = np.concatenate([betas, betas], axis=0).astype(np.float32)

    cw = g["conv_w"]

    def w_rk(r, base):
        fr0, fr1 = 128 * r, min(128 * r + 127, 1439)
        W = np.zeros((128, 128))
        for p in range(128):
            gg = base + p
            if gg >= 1008:
                continue
            sp, ci = gg // 24, gg % 24
            for f in range(fr0, fr1 + 1):
                so, co = f // 36, f % 36
                tap = sp - so
                if 0 <= tap <= 2:
                    W[p, f - fr0] = cw[co, ci, tap]
        return W

    pairs = _conv_pairs()
    cvb = np.zeros((128, len(pairs), 2, 128))
    for i, (r, k, valid, base) in enumerate(pairs):
        for h in valid:
            cvb[:, i, h, :] = w_rk(r, base + 128 * h)
    c["convw_dr"] = cvb.astype(F8NP)
    bias = np.zeros((128, NFT))
    for r in range(NFT):
        for p in range(128):
            f = 128 * r + p
            if f < 1440:
                bias[p, r] = g["conv_b"][f % 36]
    c["conv_bias"] = bias.astype(np.float32)

    e1w = g["enc1_w"]
    e1 = np.zeros((128, NFT, 128))
    for r in range(NFT):
        for p in range(128):
            f = 128 * r + p
            if f < 1440:
                e1[p, r, :] = e1w[:, (f % 36) * 40 + f // 36]
    c["enc1_w"] = e1.astype(f16)
    c["enc1_bias"] = g["enc1_b"][:, None].astype(np.float32)
    c["enc2_w"] = g["enc2_w"].T.astype(f16)

    R1, br1 = g["reg1_w"], g["reg1_b"]
    R2, br2 = g["reg2_w"], g["reg2_b"]

    def bd64(X):
        o = np.zeros((128, 64))
        o[0:64, 0:32] = X
        o[64:128, 32:64] = X
        return o

    c["r1y"] = bd64(R1.T).astype(f16)
    c["r1u"] = bd64((dt / 6) * (R1 @ W2).T).astype(f16)
    bias_r = (R1 @ (steps * dt * b2 + e2b) + br1)[:, None]
    c["bias_r"] = np.tile(bias_r, (4, 1)).astype(np.float32)
    r2bd = np.zeros((128, 4))
    for b in range(4):
        r2bd[32 * b: 32 * b + 32, b] = R2[0]
    c["r2"] = r2bd.astype(f16)
    c["br2"] = np.full((128, 1), br2[0], np.float32)
    return c


NPAIR = len(_conv_pairs())
CONST_SPECS = [
    ("convw_dr", [128, NPAIR, 2, 128], F8),
    ("A", [128, 128], F16),
    ("C", [128, 128], F16),
    ("D", [128, 128], F16),
    ("negC", [128, 128], F16),
    ("w1t", [128, 128], F16),
    ("enc1_w", [128, NFT, 128], F16),
    ("enc2_w", [128, 64], F16),
    ("r1y", [128, 64], F16),
    ("r1u", [128, 64], F16),
    ("r2", [128, 4], F16),
    ("betas", [128, 3 * N_STEPS], F32),
    ("conv_bias", [128, NFT], F32),
    ("enc1_bias", [128, 1], F32),
    ("bias_r", [128, 1], F32),
    ("br2", [128, 1], F32),
]


def _blob_layout():
    off = {F16: 0, F32: 0, F8: 0}
    lay = {}
    for n, sh, dt in CONST_SPECS:
        cols = int(np.prod(sh[1:]))
        lay[n] = (dt, off[dt], cols, sh)
        off[dt] += cols
    return lay, off[F16], off[F32], off[F8]


def pack_consts(c):
    lay, n16, n32, n8 = _blob_layout()
    blobs = {F16: np.zeros((128, n16), np.float16),
             F32: np.zeros((128, n32), np.float32),
             F8: np.zeros((128, n8), F8NP)}
    for n, (dt, off, cols, sh) in lay.items():
        arr = c[n].reshape(sh[0], cols)
        blobs[dt][: sh[0], off: off + cols] = arr
    return blobs[F16], blobs[F32], blobs[F8]


def build_nc(steps=N_STEPS, n_passes=1, stage="full"):
    # stage: timing ablations — "dma" (x load only), "conv" (+conv matmuls),
    # "silu" (+SiLU), "enc" (+enc1/enc2, no ODE/reg), "full".
    nc = bacc.Bacc("TRN2", target_bir_lowering=False)
    pairs = _conv_pairs()
    pairs_by_r = {}
    for i, (r, k, _valid, _base) in enumerate(pairs):
        pairs_by_r.setdefault(r, []).append((i, k))

    def mm(out, lhsT, rhs, start, stop):
        n = rhs.shape[-1]
        for i in range(0, n, 512):
            nc.tensor.matmul(out[:, i: i + 512], lhsT, rhs[:, i: i + 512],
                             start=start, stop=stop, skip_group_check=True)

    def mm_dr(out, w4, xt, k, start, stop):
        # fp8 DoubleRow: chunks (k, k+1) contracted in one pass
        for i in range(0, 1024, 512):
            nc.tensor.matmul(out[:, i: i + 512], w4,
                             xt[:, k: k + 2, i: i + 512],
                             start=start, stop=stop, perf_mode=DR,
                             skip_group_check=True)

    x_in = nc.dram_tensor("xp", [NSG, 128, NCKT, 1024], F8, kind="ExternalInput")
    out_t = nc.dram_tensor("out", [BPC], F32, kind="ExternalOutput")
    lay, n16, n32, n8 = _blob_layout()
    cb16_in = nc.dram_tensor("cb16", [128, n16], F16, kind="ExternalInput")
    cb32_in = nc.dram_tensor("cb32", [128, n32], F32, kind="ExternalInput")
    cb8_in = nc.dram_tensor("cb8", [128, n8], F8, kind="ExternalInput")

    with TileContext(nc) as tc:
        import contextlib
        es = contextlib.ExitStack()
        with es:
            cpool = es.enter_context(tc.tile_pool(name="consts", bufs=1))
            big = es.enter_context(tc.tile_pool(name="big", bufs=1))

            cb16 = cpool.tile([128, n16], F16, tag="cb16", name="cb16")
            cb32 = cpool.tile([128, n32], F32, tag="cb32", name="cb32")
            cb8 = cpool.tile([128, n8], F8, tag="cb8", name="cb8")
            nc.sync.dma_start(out=cb8[:], in_=cb8_in[:])
            nc.sync.dma_start(out=cb32[:], in_=cb32_in[:])
            nc.sync.dma_start(out=cb16[:], in_=cb16_in[:])
            blobs = {F16: cb16, F32: cb32, F8: cb8}
            ct = {}
            for n, (dt, off, cols, sh) in lay.items():
                v = blobs[dt][: sh[0], off: off + cols]
                if len(sh) == 3:
                    v = v.rearrange("p (a b) -> p a b", b=sh[2])
                elif len(sh) == 4:
                    v = v.rearrange("p (a b c) -> p a b c", b=sh[2], c=sh[3])
                ct[n] = v

            if stage in ("enc", "full"):
                y0 = big.tile([128, HB], F16, tag="y0")
            if stage == "full":
                t1 = big.tile([128, HB], F16, tag="t1")
                t2 = big.tile([128, HB], F16, tag="t2")
                t3 = big.tile([128, HB], F16, tag="t3")
                U = [big.tile([128, HB], F16, tag=f"U{n}", name=f"U{n}")
                     for n in range(steps)]
                pred_sb = big.tile([128, HB // 2], F32, tag="pred")

            # Static PSUM layout (8 banks exactly), manually double-buffered.
            # Pool-rotated PSUM tiles route every bank handoff through SP
            # TileRelease/alloc semaphores, which serialized conv against
            # silu; static tiles leave only region-level data deps.
            psum = es.enter_context(
                tc.tile_pool(name="psum", bufs=1, space="PSUM"))
            if stage != "dma":
                cvps = [psum.tile([128, 1024], F32, tag="cvpsA", name="cvpsA"),
                        psum.tile([128, 1024], F32, tag="cvpsB", name="cvpsB")]


            # ------- fused conv+encoder with interleaved per-wave ODE -------
            # sg order pairs (v, 4+v): wave v's y0 halves complete every 2 sgs,
            # and its ODE chain is drip-fed into the emission stream.
            with tc.tile_pool(name="p1h", bufs=3) as sbh, \
                 tc.tile_pool(name="p1_sb", bufs=3) as sb1, \
                 tc.tile_pool(name="en_ps", bufs=1, space="PSUM") as eps, \
                 tc.tile_pool(name="ode_ps", bufs=1, space="PSUM") as ops, \
                 tc.tile_pool(name="ode_sb", bufs=4) as osb, \
                 tc.tile_pool(name="rg_sb", bufs=2) as rsb:
              b = ct["betas"]
              tail = {}
              pending = []
              wstate = {}

              def emit_out_dma():
                  pv = pred_sb.rearrange("p (q n) -> p q n", n=512)
                  ov = out_t.rearrange("(h q par n) -> h par q n",
                                       h=2, par=2, n=512)
                  for k2, (hh, par) in enumerate(
                          [(0, 0), (1, 0), (0, 1), (1, 1)]):
                      nc.sync.dma_start(out=ov[hh, par],
                                        in_=pv[k2: k2 + 1, 0: NCH // 2, :])

              def enc1mm(ept, ht, rr, start, stop):
                  # k-tile 11 holds only 32 real rows (features 1408-1439;
                  # weight rows 32-127 are zero) — use K=32 there
                  kk = 32 if rr == NFT - 1 else 128
                  mm(ept, ct["enc1_w"][0:kk, rr, :], ht[0:kk, rr, :],
                     start, stop)

              for _pass in range(n_passes):

                def emit_tail(sg):
                    # relu/enc2/y0-copy for super-group sg (deferred)
                    ep, e1t, _h = tail.pop(sg)
                    ro = 64 * (sg // 4)
                    cols = bass.ts(sg % 4, 1024)
                    nc.vector.tensor_scalar(
                        out=e1t[:], in0=ep[:], scalar1=ct["enc1_bias"][:],
                        scalar2=0.0, op0=ALU.add, op1=ALU.max)
                    tp = eps.tile([128, 1024], F32, tag="ep", name="tp")
                    mm(tp[0:64, :], ct["enc2_w"][:], e1t[:], True, True)
                    nc.vector.tensor_copy(out=y0[ro: ro + 64, cols],
                                          in_=tp[0:64, :])

                def queue_wave_split(v):
                    # column-split chain for the last wave: two interleaved
                    # 512-col half-chains to halve the exposed tail latency
                    n = 0
                    wt = {}

                    def c_init():
                        wt[0] = ops.tile([128, 1024], F32, tag="w", name="w3")
                        mm(wt[0], ct["w1t"][:], y0[:, bass.ts(v, 1024)],
                           True, False)

                    def halfops(hw):
                        wc = bass.ds(1024 * v + 512 * hw, 512)
                        pc = bass.ds(512 * hw, 512)

                        def h_t1():
                            nc.scalar.activation(t1[:, wc], wt[0][:, pc],
                                                 AF.Tanh, bias=b[:, 0:1])

                        def h_A1():
                            nc.tensor.matmul(wt[0][:, pc], ct["A"][:],
                                             t1[:, wc], start=False,
                                             stop=False, skip_group_check=True)

                        def h_t2():
                            nc.scalar.activation(t2[:, wc], wt[0][:, pc],
                                                 AF.Tanh, bias=b[:, 1:2])

                        def h_d32():
                            d = osb.tile([128, 512], F16, tag="dh", name="dh")
                            wt[(hw, "d")] = d
                            nc.vector.tensor_sub(out=d[:], in0=t2[:, wc],
                                                 in1=t1[:, wc])

                        def h_A2():
                            nc.tensor.matmul(wt[0][:, pc], ct["A"][:],
                                             wt[(hw, "d")][:], start=False,
                                             stop=False, skip_group_check=True)

                        def h_t3():
                            nc.scalar.activation(t3[:, wc], wt[0][:, pc],
                                                 AF.Tanh, bias=b[:, 1:2])

                        def h_d43():
                            d2 = osb.tile([128, 512], F16, tag="dh", name="dh2")
                            wt[(hw, "d")] = d2
                            nc.vector.scalar_tensor_tensor(
                                out=d2[:], in0=t2[:, wc], scalar=-0.5,
                                in1=t3[:, wc], op0=ALU.mult, op1=ALU.add)

                        def h_C():
                            nc.tensor.matmul(wt[0][:, pc], ct["C"][:],
                                             wt[(hw, "d")][:], start=False,
                                             stop=True, skip_group_check=True)

                        def h_t4():
                            nc.scalar.activation(U[n][:, wc], wt[0][:, pc],
                                                 AF.Tanh, bias=b[:, 2:3])

                        def h_u1():
                            nc.vector.tensor_add(out=U[n][:, wc],
                                                 in0=U[n][:, wc],
                                                 in1=t1[:, wc])

                        def h_u2():
                            nc.vector.scalar_tensor_tensor(
                                out=U[n][:, wc], in0=t2[:, wc], scalar=2.0,
                                in1=U[n][:, wc], op0=ALU.mult, op1=ALU.add)

                        def h_u3():
                            nc.vector.scalar_tensor_tensor(
                                out=U[n][:, wc], in0=t3[:, wc], scalar=2.0,
                                in1=U[n][:, wc], op0=ALU.mult, op1=ALU.add)

                        def h_reg():
                            # idx=hw: rows 64*hw of the shared rp region
                            cc = bass.ts(2 * v + hw, 512)
                            orow = slice(64 * hw, 64 * hw + 64)
                            tp_ = (0, 64 * hw)
                            nc.tensor.matmul(
                                wt[0][orow, 0:512], ct["r1y"][:], y0[:, cc],
                                start=True, stop=False, tile_position=tp_,
                                skip_group_check=True)
                            nc.tensor.matmul(
                                wt[0][orow, 0:512], ct["r1u"][:], U[n][:, cc],
                                start=False, stop=True, tile_position=tp_,
                                skip_group_check=True)

                        return [h_t1, h_A1, h_t2, h_d32, h_A2, h_t3, h_d43,
                                h_C, h_t4, h_u1, h_u2, h_u3, h_reg]

                    def c_finish():
                        rr = rsb.tile([128, 512], F16, tag="rr", name="rr")
                        nc.vector.tensor_scalar(
                            out=rr[:], in0=wt[0][:, 0:512],
                            scalar1=ct["bias_r"][:], scalar2=0.0,
                            op0=ALU.add, op1=ALU.max)
                        nc.tensor.matmul(wt[0][0:4, 512:1024], ct["r2"][:],
                                         rr[:], start=True, stop=True,
                                         skip_group_check=True)
                        nc.vector.tensor_scalar_add(
                            out=pred_sb[0:4, bass.ts(v, 512)],
                            in0=wt[0][0:4, 512:1024],
                            scalar1=ct["br2"][0:4])

                    seq = [c_init]
                    ha, hb = halfops(0), halfops(1)
                    for x, y_ in zip(ha, hb):
                        seq.extend([x, y_])
                    seq.append(c_finish)
                    pending.extend(seq)

                def queue_wave(v):
                    wc = bass.ts(v, 1024)
                    n = 0  # single RK4 step

                    def c_init():
                        wt = ops.tile([128, 1024], F32, tag="w", name="w")
                        wstate[v] = wt
                        mm(wt, ct["w1t"][:], y0[:, wc], True, False)

                    def c_t1():
                        nc.scalar.activation(t1[:, wc], wstate[v][:], AF.Tanh,
                                             bias=b[:, 0:1])

                    def c_A1():
                        mm(wstate[v], ct["A"][:], t1[:, wc], False, False)

                    def c_t2():
                        nc.scalar.activation(t2[:, wc], wstate[v][:], AF.Tanh,
                                             bias=b[:, 1:2])

                    def c_d32():
                        d = osb.tile([128, 1024], F16, tag="d", name="d")
                        wstate[(v, "d")] = d
                        nc.vector.tensor_sub(out=d[:], in0=t2[:, wc],
                                             in1=t1[:, wc])

                    def c_A2():
                        mm(wstate[v], ct["A"][:], wstate[(v, "d")][:],
                           False, False)

                    def c_t3():
                        nc.scalar.activation(t3[:, wc], wstate[v][:], AF.Tanh,
                                             bias=b[:, 1:2])

                    def c_d43():
                        d2 = osb.tile([128, 1024], F16, tag="d", name="d2")
                        wstate[(v, "d")] = d2
                        nc.vector.scalar_tensor_tensor(
                            out=d2[:], in0=t2[:, wc], scalar=-0.5,
                            in1=t3[:, wc], op0=ALU.mult, op1=ALU.add)

                    def c_C():
                        mm(wstate[v], ct["C"][:], wstate[(v, "d")][:],
                           False, True)

                    def c_t4():
                        nc.scalar.activation(U[n][:, wc], wstate[v][:],
                                             AF.Tanh, bias=b[:, 2:3])

                    def c_u1():
                        nc.vector.tensor_add(out=U[n][:, wc], in0=U[n][:, wc],
                                             in1=t1[:, wc])

                    def c_u2():
                        nc.vector.scalar_tensor_tensor(
                            out=U[n][:, wc], in0=t2[:, wc], scalar=2.0,
                            in1=U[n][:, wc], op0=ALU.mult, op1=ALU.add)

                    def c_u3():
                        nc.vector.scalar_tensor_tensor(
                            out=U[n][:, wc], in0=t3[:, wc], scalar=2.0,
                            in1=U[n][:, wc], op0=ALU.mult, op1=ALU.add)

                    def c_reg():
                        rp = ops.tile([128, 1024], F32, tag="w",
                                      name=f"rp{v}")
                        for idx in range(2):
                            cc = bass.ts(2 * v + idx, 512)
                            orow = slice(64 * idx, 64 * idx + 64)
                            tp_ = (0, 64 * idx)
                            nc.tensor.matmul(
                                rp[orow, 0:512], ct["r1y"][:], y0[:, cc],
                                start=True, stop=False, tile_position=tp_,
                                skip_group_check=True)
                            nc.tensor.matmul(
                                rp[orow, 0:512], ct["r1u"][:], U[n][:, cc],
                                start=False, stop=True, tile_position=tp_,
                                skip_group_check=True)
                        rr = rsb.tile([128, 512], F16, tag="rr", name="rr")
                        nc.vector.tensor_scalar(
                            out=rr[:], in0=rp[:, 0:512],
                            scalar1=ct["bias_r"][:], scalar2=0.0,
                            op0=ALU.add, op1=ALU.max)
                        nc.tensor.matmul(rp[0:4, 512:1024], ct["r2"][:],
                                         rr[:], start=True, stop=True,
                                         skip_group_check=True)
                        nc.vector.tensor_scalar_add(
                            out=pred_sb[0:4, bass.ts(v, 512)],
                            in0=rp[0:4, 512:1024],
                            scalar1=ct["br2"][0:4])

                    pending.extend([c_init, c_t1, c_A1, c_t2, c_d32, c_A2,
                                    c_t3, c_d43, c_C, c_t4, c_u1, c_u2,
                                    c_u3, c_reg])

                sg_order = [0, 4, 1, 5, 2, 6, 3, 7]
                for idx, sg in enumerate(sg_order):
                    xt = sb1.tile([128, NCKT, 1024], F8, tag="xt", name="xt")
                    if idx == 0 and _pass == 0:
                        # split the very first DMA so conv tile 0 (chunks
                        # 0-1) starts as soon as possible; later passes are
                        # pipelined and don't need the extra instruction
                        nc.sync.dma_start(out=xt[:, 0:2, :],
                                          in_=x_in[sg][:, 0:2, :])
                        nc.sync.dma_start(out=xt[:, 2:NCKT, :],
                                          in_=x_in[sg][:, 2:NCKT, :])
                    else:
                        nc.sync.dma_start(out=xt[:], in_=x_in[sg])
                    if stage not in ("dma", "conv"):
                        h = sbh.tile([128, NFT, 1024], F16, tag="h", name="h")
                    if stage in ("enc", "full"):
                        ep = eps.tile([128, 1024], F32, tag="ep", name="ep")
                        e1t = sb1.tile([128, 1024], F16, tag="e1", name="e1t")
                        tail[sg] = (ep, e1t, h)

                    def conv(r, xt=xt):
                        cp = cvps[r % 2]
                        prs = pairs_by_r[r]
                        for j, (widx, k) in enumerate(prs):
                            mm_dr(cp, ct["convw_dr"][:, widx], xt, k,
                                  j == 0, j == len(prs) - 1)

                    # Emission order within a slot: silu r first, then the
                    # next conv (its PSUM bank is already free), then cover
                    # work (enc1 / ODE pops).  Putting conv BEFORE the cover
                    # made the in-order PE stream stall head-of-line on the
                    # bank freed by silu r-2, with runnable cover work queued
                    # behind the stall — exposing ~0.3-0.5us per slot.
                    if stage != "dma":
                        conv(0)
                        conv(1)
                    for r in range(NFT):
                        if stage in ("dma", "conv"):
                            if stage == "conv" and r + 2 < NFT:
                                conv(r + 2)
                            continue
                        nc.scalar.activation(h[:, r, :], cvps[r % 2][:],
                                             AF.Silu,
                                             bias=ct["conv_bias"][:, r: r + 1])
                        if r + 2 < NFT:
                            conv(r + 2)
                        if stage == "silu":
                            continue
                        if r <= 1 and len(tail) > 1:
                            # previous sg's enc1 tail (rr=10,11) emitted as
                            # cover work in this sg's first two slots; then
                            # its relu/enc2/y0 and (cross-pass) wave/out-DMA
                            prev = sg_order[idx - 1]
                            ep_p, _e1t_p, h_p = tail[prev]
                            rr = NFT - 2 + r
                            enc1mm(ep_p, h_p, rr, False, rr == NFT - 1)
                            if r == 1:
                                emit_tail(prev)
                                if stage == "full" and prev >= 4:
                                    queue_wave(prev - 4)
                                    if prev == 7:
                                        pending.append(emit_out_dma)
                        if r >= 2:
                            rr = r - 2
                            enc1mm(ep, h, rr, rr == 0, False)
                        if pending:
                            pending.pop(0)()
              if stage not in ("dma", "conv", "silu"):
                last = sg_order[-1]
                ep_l, _e1t_l, h_l = tail[last]
                for rr in (NFT - 2, NFT - 1):
                    enc1mm(ep_l, h_l, rr, False, rr == NFT - 1)
                emit_tail(last)
              if stage == "full":
                queue_wave_split(3)
              while pending:
                  pending.pop(0)()
              if stage == "full":
                emit_out_dma()
              if stage != "full":
                # keep the ExternalOutput written (timing-only variants)
                ov2 = out_t.rearrange("(a b) -> a b", b=16)
                nc.sync.dma_start(out=ov2[0:1], in_=cb32[0:1, 0:16])
    nc.compile()
    return nc


_CACHE = {}


def _get_nc(steps=N_STEPS, n_passes=1, stage="full"):
    key = (steps, n_passes, stage)
    if key not in _CACHE:
        _CACHE[key] = build_nc(steps, n_passes, stage)
    return _CACHE[key]


def prep_x(x):
    B = x.shape[0]
    xpad = np.zeros((B, 42, 24), F8NP)
    xpad[:, 1:41, :] = np.asarray(x, np.float32).astype(F8NP)
    flat = xpad.reshape(B, 1008)
    padded = np.zeros((B, 1024), F8NP)
    padded[:, :1008] = flat
    # main 8 chunks (rows 0-1023) + 64-offset chunks (rows 192-703)
    rows = np.concatenate(
        [padded] + [padded[:, ob: ob + 128] for ob in OFF_BASES], axis=1)
    out = []
    for c in range(N_CORES):
        xc = rows[c * BPC: (c + 1) * BPC].reshape(NSG, 1024, 128 * NCKT)
        xt = np.ascontiguousarray(np.transpose(xc, (0, 2, 1)))
        out.append(xt.reshape(NSG, NCKT, 128, 1024).transpose(0, 2, 1, 3))
    return [np.ascontiguousarray(o) for o in out]


def make_in_maps(inputs):
    consts = make_consts(inputs)
    b16, b32, b8 = pack_consts(consts)
    xs = prep_x(np.asarray(inputs["x"]))
    return [{"cb16": b16, "cb32": b32, "cb8": b8, "xp": xs[i]}
            for i in range(N_CORES)]


def kernel(**inputs):
    in_maps = make_in_maps(inputs)
    nc = _get_nc()
    res = run_bass_kernel_spmd(nc, in_maps, list(range(N_CORES)))
    return np.concatenate([res.results[i]["out"] for i in range(N_CORES)])

